# revision 11
# baseline (speedup 1.0000x reference)
"""CfC RNN kernel for Trainium2 (8 NeuronCores, batch-data-parallel).

Model (per step, reference semantics, ts = 1.0):
    z_in = concat([x_t, h])                      # [B, I+H] = [B, 768]
    z1 = 1.7159*tanh(0.666*(z_in @ wb1.T + bb1)) # [B, 1024]
    z2 = 1.7159*tanh(0.666*(z1 @ wb2.T + bb2))   # [B, 1024]
    ff1 = tanh(z2 @ wff1.T + bff1)               # [B, 512]
    ff2 = tanh(z2 @ wff2.T + bff2)
    t   = sigmoid(z2 @ (wta+wtb).T + (bta+btb))  # ta*1+tb folded
    h'  = ff1 + t*(ff2-ff1)

Device layout is dim-major everywhere: [dim -> 128 partitions, batch -> free].
Algebraic folds (host-side):
  - store z1' = tanh(0.666*pre1)  (the 1.7159 is folded into wb2)
  - store z2' = tanh(0.666*pre2)  (the 1.7159 is folded into the head weights)
  - t_a*ts + t_b with ts=1 == one matmul with (wta+wtb), bias (bta+btb)
"""

import sys

sys.path.insert(0, "/opt/trn_rl_repo")

import numpy as np

import concourse.bass as bass
import concourse.tile as tile
from concourse import bacc, mybir
from concourse import bass_utils
from concourse.bass import ds, ts

B, T, I, H, BU = 64, 512, 256, 512, 1024
NCORES = 8
BC = B // NCORES  # batch rows per core
KX = I // 128     # 2  x K-chunks
KH = H // 128     # 4  h K-chunks
M1 = BU // 128    # 8  mm1 out tiles
K2 = BU // 128    # 8  mm2 K-chunks
M2 = BU // 128    # 8  mm2 out tiles
MH = H // 128     # 4  head out tiles

AF = mybir.ActivationFunctionType

# --- build configuration ---------------------------------------------------
CFG = dict(
    dtype_w="bfloat16",  # weights dtype (stationary operand)
    dtype_a="bfloat16",  # activations/x/h dtype (moving operand)
    unroll=2,           # steps per For_i iteration
    hints=False,        # hint_engines on the loop back-edge (CRASHES device!)
    staggered=True,     # staggered_reset loop semaphore recycling
    # timing-knockout flags (break correctness; for diagnosis only)
    ko_dma=False,       # drop the per-step ys DMA
    ko_dyn=False,       # static x index instead of ds(t)
    ko_act=False,       # skip ACT + DVE (PE only)
    ko_mm=False,        # skip matmuls (ACT/DVE only)
    ko_all=False,       # nearly-empty loop body
    onetable=True,      # express sigmoid via tanh => single ACT table set
    outer=1,            # whole-kernel repetitions (timing amplifier)
    wide=True,          # single wide ACT/DVE per phase (requires zero biases)
    preu=True,          # device-precompute u = 0.666*(x @ w1x.T); bf16 only
    wsplit=False,       # hi/lo split weights: W = bf16(W) + bf16(W - bf16(W))
    asplit=False,       # hi/lo split activations (z1/z2/h/x); wide mode only
    ldwopt=False,       # pass --enable-ldw-opt=true to walrus (dedups LDWEIGHTS)
    abufs=2,            # acts tile-pool buffers
    pbufs=2,            # psum tile-pool buffers
    static=False,       # python-unrolled T loop (for TimelineSim / small T)
    wag=True,           # upload 1/8 of weights per core + on-device AllGather
    ys16=True,          # ys output in bf16 (DMA the bf16 h tile directly)
)

# flat per-tensor weight column counts (x128 partitions, bf16)
_WCOLS = [("w1", 6 * BU), ("w2", K2 * BU), ("wf1", K2 * H), ("wf2", K2 * H),
          ("wt", K2 * H)]
WTOT = sum(c for _, c in _WCOLS)          # 26624
WSH = WTOT // NCORES                      # 3328 cols per core shard


_LDWOPT_PATCHED = False


def _patch_ldwopt():
    global _LDWOPT_PATCHED
    if _LDWOPT_PATCHED:
        return
    _LDWOPT_PATCHED = True
    orig = bass_utils.run_command

    def patched(cmd, *a, **kw):
        if isinstance(cmd, list):
            cmd = ["--enable-ldw-opt=true" if c == "--enable-ldw-opt=false" else c
                   for c in cmd]
        return orig(cmd, *a, **kw)

    bass_utils.run_command = patched


def _dt(name):
    return {"float32": mybir.dt.float32, "bfloat16": mybir.dt.bfloat16,
            "float32r": mybir.dt.float32r}[name]


def build(T_steps=T, cfg=CFG):
    DTW = _dt(cfg["dtype_w"])
    DT = _dt(cfg["dtype_a"])
    nc = bacc.Bacc("TRN2", target_bir_lowering=False, debug=False,
                   num_devices=NCORES)

    f32 = mybir.dt.float32
    NA = 2 if cfg["asplit"] else 1
    xT_d = nc.dram_tensor("xT", [128, T, KX, NA * BC], DT, kind="ExternalInput").ap()
    NW = 2 if cfg["wsplit"] else 1
    wag = cfg.get("wag", False)
    if wag:
        assert NW == 1
        wsh_d = nc.dram_tensor("wsh", [128, WSH], DTW, kind="ExternalInput").ap()
    else:
        w1_d = nc.dram_tensor("w1", [128, NW, KX + KH, BU], DTW, kind="ExternalInput").ap()
        w2_d = nc.dram_tensor("w2", [128, NW, K2, BU], DTW, kind="ExternalInput").ap()
        wf1_d = nc.dram_tensor("wf1", [128, NW, K2, H], DTW, kind="ExternalInput").ap()
        wf2_d = nc.dram_tensor("wf2", [128, NW, K2, H], DTW, kind="ExternalInput").ap()
        wt_d = nc.dram_tensor("wt", [128, NW, K2, H], DTW, kind="ExternalInput").ap()
    bias_d = nc.dram_tensor("biases", [128, 28], f32, kind="ExternalInput").ap()
    ys16 = cfg.get("ys16", False)
    if ys16:
        assert NA == 1 and DT != f32
        ys_d = nc.dram_tensor("ys", [T, 128, KH * BC], DT, kind="ExternalOutput").ap()
    else:
        ys_d = nc.dram_tensor("ys", [T, 128, KH * BC], f32, kind="ExternalOutput").ap()

    with tile.TileContext(nc) as tc:
        with tc.tile_pool(name="weights", bufs=1) as wp, \
             tc.tile_pool(name="state", bufs=1) as sp, \
             tc.tile_pool(name="acts", bufs=cfg["abufs"]) as ap_, \
             tc.tile_pool(name="psum", bufs=cfg["pbufs"], space="PSUM") as pp:
            xT = wp.tile([128, T, KX, NA * BC], DT)
            w1 = wp.tile([128, NW, KX + KH, BU], DTW)
            w2 = wp.tile([128, NW, K2, BU], DTW)
            wf1 = wp.tile([128, NW, K2, H], DTW)
            wf2 = wp.tile([128, NW, K2, H], DTW)
            wt = wp.tile([128, NW, K2, H], DTW)
            bia = wp.tile([128, 28], f32)
            nc.sync.dma_start(xT[:], xT_d[:])
            nc.sync.dma_start(bia[:], bias_d[:])
            if wag:
                with tc.tile_pool(name="wag_dram", bufs=1, space="DRAM") as dram:
                    wag_in = dram.tile([128, WSH], DTW)
                    wag_out = dram.tile([NCORES * 128, WSH], DTW)
                    nc.gpsimd.dma_start(wag_in[:], wsh_d[:])
                    nc.gpsimd.collective_compute(
                        "AllGather", mybir.AluOpType.bypass,
                        replica_groups=[list(range(NCORES))],
                        ins=[wag_in.opt()], outs=[wag_out.opt()])
                    wag3 = wag_out.rearrange("(c p) s -> c p s", c=NCORES)
                    flats = {
                        "w1": w1.rearrange("p a k m -> p (a k m)"),
                        "w2": w2.rearrange("p a k m -> p (a k m)"),
                        "wf1": wf1.rearrange("p a k m -> p (a k m)"),
                        "wf2": wf2.rearrange("p a k m -> p (a k m)"),
                        "wt": wt.rearrange("p a k m -> p (a k m)"),
                    }
                    for c in range(NCORES):
                        off = 0
                        for nm, cols in _WCOLS:
                            sl = cols // NCORES
                            nc.sync.dma_start(
                                flats[nm][:, c * sl:(c + 1) * sl],
                                wag3[c, :, off:off + sl])
                            off += sl
            else:
                for sb_t, dr in ((w1, w1_d), (w2, w2_d), (wf1, wf1_d),
                                 (wf2, wf2_d), (wt, wt_d)):
                    nc.sync.dma_start(sb_t[:], dr[:])

            h = sp.tile([128, KH, NA * BC], DT)  # recurrent state, dim-major
            h32 = sp.tile([128, KH * BC], f32)   # fp32 copy for output DMA
            if DT == f32 and not cfg["asplit"]:
                h = h32.rearrange("p (c b) -> p c b", c=KH)

            u_sb = None
            if cfg["preu"]:
                u_sb = wp.tile([128, T_steps, M1 * BC], DT)
                TB = 512 // BC                   # time-block per psum bank
                for m in range(M1):
                    for nb in range((T_steps + TB - 1) // TB):
                        nsz = min(TB, T_steps - nb * TB)
                        pu = pp.tile([128, TB * BC], f32, tag="pu")
                        for p in range(NW):
                            for k in range(KX):
                                for a in range(NA):
                                    nc.tensor.matmul(
                                        pu[:, 0:nsz * BC],
                                        w1[:, p, k, ts(m, 128)],
                                        xT[:, nb * TB:nb * TB + nsz, k, ts(a, BC)],
                                        start=(p == 0 and k == 0 and a == 0),
                                        stop=(p == NW - 1 and k == KX - 1
                                              and a == NA - 1))
                        nc.scalar.activation(
                            u_sb[:, nb * TB:nb * TB + nsz, ts(m, BC)],
                            pu[:, 0:nsz * BC], AF.Copy, bias=0.0, scale=0.666)

            def step(t_idx):
                if cfg["ko_all"]:
                    d0 = ap_.tile([128, BC], f32, tag="d0")
                    nc.vector.memset(d0[:], 0.0)
                    return
                x_idx = ds(t_idx, 1) if not cfg["ko_dyn"] else ds(0, 1)
                if not cfg["ko_act"]:
                    z1 = ap_.tile([128, M1, NA * BC], DT, tag="z1")
                    z2 = ap_.tile([128, M2, NA * BC], DT, tag="z2")
                    if cfg["asplit"]:
                        z1f = ap_.tile([128, M1 * BC], f32, tag="z1f")
                        z2f = ap_.tile([128, M2 * BC], f32, tag="z2f")
                    ff1 = ap_.tile([128, MH * BC], f32, tag="ff1")
                    ff2 = ap_.tile([128, MH * BC], f32, tag="ff2")
                    tt = ap_.tile([128, MH * BC], f32, tag="tt")
                else:
                    z1 = z2 = h
                    ff1 = ff2 = tt = h
                if not cfg["ko_mm"]:
                    z1ps = pp.tile([128, M1 * BC], f32, tag="z1ps")
                    z2ps = pp.tile([128, M2 * BC], f32, tag="z2ps")
                    hps = pp.tile([128, 3 * MH * BC], f32, tag="hps")

                # ---- mm1: z1pre = [x_t; h] @ wb1.T  (K = 2 x-chunks + 4 h-chunks)
                for m in range(M1 if not cfg["ko_mm"] else 0):
                    first = True
                    for p in range(NW):
                        if not cfg["preu"]:
                            for k in range(KX):
                                for a in range(NA):
                                    nc.tensor.matmul(
                                        z1ps[:, ts(m, BC)],
                                        w1[:, p, k, ts(m, 128)],
                                        xT[:, x_idx, k, ts(a, BC)],
                                        start=first, stop=False)
                                    first = False
                        for k in range(KH):
                            for a in range(NA):
                                nc.tensor.matmul(
                                    z1ps[:, ts(m, BC)],
                                    w1[:, p, KX + k, ts(m, 128)],
                                    h[:, k, ts(a, BC)],
                                    start=first,
                                    stop=(p == NW - 1 and k == KH - 1
                                          and a == NA - 1))
                                first = False
                # z1 = tanh(0.666*pre + 0.666*bb1)
                def split_phase(dst, dstf, src_ps, scale):
                    # dst: [128, M, 2*BC] hi/lo bf16; dstf: [128, M*BC] f32
                    M = dst.shape[1]
                    dst3 = dst.rearrange("p m (a b) -> p m a b", a=NA)
                    nc.scalar.activation(dstf[:], src_ps[:], AF.Tanh, scale=scale)
                    dfv = dstf.rearrange("p (m b) -> p m b", m=M)
                    nc.scalar.activation(dst3[:, :, 0, :], dfv[:], AF.Copy)
                    nc.vector.tensor_sub(dst3[:, :, 1, :], dfv[:], dst3[:, :, 0, :])

                if cfg["preu"] and not cfg["ko_act"]:
                    zpre = ap_.tile([128, M1 * BC], f32, tag="zpre")
                    nc.vector.scalar_tensor_tensor(
                        zpre[:], z1ps[:], 0.666, u_sb[:, ds(t_idx, 1), :],
                        mybir.AluOpType.mult, mybir.AluOpType.add)
                    if cfg["asplit"]:
                        split_phase(z1, z1f, zpre, 1.0)
                    else:
                        nc.scalar.activation(z1.rearrange("p m b -> p (m b)"),
                                             zpre[:], AF.Tanh)
                elif cfg["wide"] and not cfg["ko_act"]:
                    if cfg["asplit"]:
                        split_phase(z1, z1f, z1ps, 0.666)
                    else:
                        nc.scalar.activation(z1.rearrange("p m b -> p (m b)"),
                                             z1ps[:], AF.Tanh, scale=0.666)
                else:
                    for m in range(M1 if not cfg["ko_act"] else 0):
                        nc.scalar.activation(z1[:, m, :], z1ps[:, ts(m, BC)],
                                             AF.Tanh, bias=bia[:, m:m + 1], scale=0.666)

                # ---- mm2: z2pre = z1 @ (1.7159*wb2).T
                for m in range(M2 if not cfg["ko_mm"] else 0):
                    for p in range(NW):
                        for k in range(K2):
                            for a in range(NA):
                                nc.tensor.matmul(
                                    z2ps[:, ts(m, BC)],
                                    w2[:, p, k, ts(m, 128)],
                                    z1[:, k if not cfg["ko_act"] else k % KH,
                                       ts(a, BC)],
                                    start=(p == 0 and k == 0 and a == 0),
                                    stop=(p == NW - 1 and k == K2 - 1
                                          and a == NA - 1))
                if cfg["wide"] and not cfg["ko_act"]:
                    if cfg["asplit"]:
                        split_phase(z2, z2f, z2ps, 0.666)
                    else:
                        nc.scalar.activation(z2.rearrange("p m b -> p (m b)"),
                                             z2ps[:], AF.Tanh, scale=0.666)
                else:
                    for m in range(M2 if not cfg["ko_act"] else 0):
                        nc.scalar.activation(z2[:, m, :], z2ps[:, ts(m, BC)],
                                             AF.Tanh, bias=bia[:, 8 + m:9 + m], scale=0.666)

                # ---- heads: ff1, ff2, t (weights pre-scaled by 1.7159)
                for hd, w_sb in enumerate((wf1, wf2, wt) if not cfg["ko_mm"] else ()):
                    for m in range(MH):
                        for p in range(NW):
                            for k in range(K2):
                                for a in range(NA):
                                    nc.tensor.matmul(
                                        hps[:, ts(hd * MH + m, BC)],
                                        w_sb[:, p, k, ts(m, 128)],
                                        z2[:, k if not cfg["ko_act"] else k % KH,
                                           ts(a, BC)],
                                        start=(p == 0 and k == 0 and a == 0),
                                        stop=(p == NW - 1 and k == K2 - 1
                                              and a == NA - 1))
                if cfg["wide"] and not cfg["ko_act"]:
                    # ff1|ff2 in one tanh over hps cols [0, 2*MH*BC)
                    nc.scalar.activation(ff1[:], hps[:, 0:MH * BC], AF.Tanh)
                    nc.scalar.activation(ff2[:], hps[:, MH * BC:2 * MH * BC], AF.Tanh)
                    nc.scalar.activation(tt[:], hps[:, 2 * MH * BC:3 * MH * BC],
                                         AF.Tanh, scale=0.5)
                else:
                    for m in range(MH if not cfg["ko_act"] else 0):
                        nc.scalar.activation(ff1[:, ts(m, BC)], hps[:, ts(m, BC)],
                                             AF.Tanh, bias=bia[:, 16 + m:17 + m])
                    for m in range(MH if not cfg["ko_act"] else 0):
                        nc.scalar.activation(ff2[:, ts(m, BC)], hps[:, ts(MH + m, BC)],
                                             AF.Tanh, bias=bia[:, 20 + m:21 + m])
                    for m in range(MH if not cfg["ko_act"] else 0):
                        if cfg["onetable"]:
                            nc.scalar.activation(tt[:, ts(m, BC)], hps[:, ts(2 * MH + m, BC)],
                                                 AF.Tanh, bias=bia[:, 24 + m:25 + m],
                                                 scale=0.5)
                        else:
                            nc.scalar.activation(tt[:, ts(m, BC)], hps[:, ts(2 * MH + m, BC)],
                                                 AF.Sigmoid, bias=bia[:, 24 + m:25 + m])

                # ---- h' = ff1 + t*(ff2-ff1); onetable: t = 0.5*(1+tt_raw)
                if cfg["wide"] and not cfg["ko_act"]:
                    W = MH * BC
                    d = ap_.tile([128, W], f32, tag="d")
                    e = ap_.tile([128, W], f32, tag="e")
                    nc.vector.tensor_sub(d[:], ff2[:], ff1[:])
                    nc.vector.tensor_scalar_add(e[:], tt[:], 1.0)
                    nc.vector.tensor_mul(d[:], d[:], e[:])
                    nc.vector.scalar_tensor_tensor(
                        h32[:], d[:], 0.5, ff1[:],
                        mybir.AluOpType.mult, mybir.AluOpType.add)
                for c in range(0 if (cfg["wide"] or cfg["ko_act"]) else KH):
                    d = ap_.tile([128, BC], f32, tag="d")
                    e = ap_.tile([128, BC], f32, tag="e")
                    nc.vector.tensor_sub(d[:], ff2[:, ts(c, BC)], ff1[:, ts(c, BC)])
                    if cfg["onetable"]:
                        nc.vector.tensor_scalar_add(e[:], tt[:, ts(c, BC)], 1.0)
                        nc.vector.tensor_mul(d[:], d[:], e[:])
                        nc.vector.scalar_tensor_tensor(
                            h32[:, ts(c, BC)], d[:], 0.5, ff1[:, ts(c, BC)],
                            mybir.AluOpType.mult, mybir.AluOpType.add)
                    else:
                        nc.vector.tensor_mul(e[:], d[:], tt[:, ts(c, BC)])
                        nc.vector.tensor_add(h32[:, ts(c, BC)], e[:], ff1[:, ts(c, BC)])
                if not cfg["ko_act"] and (DT != f32 or cfg["asplit"]):
                    h3 = h.rearrange("p c (a b) -> p c a b", a=NA)
                    h32v = h32.rearrange("p (c b) -> p c b", c=KH)
                    nc.scalar.activation(h3[:, :, 0, :], h32v[:], AF.Copy)
                    if cfg["asplit"]:
                        nc.vector.tensor_sub(h3[:, :, 1, :], h32v[:], h3[:, :, 0, :])

                if not cfg["ko_dma"]:
                    if ys16:
                        nc.sync.dma_start(ys_d[ds(t_idx, 1), :, :],
                                          h.rearrange("p c b -> p (c b)")[:])
                    else:
                        nc.sync.dma_start(ys_d[ds(t_idx, 1), :, :], h32[:])

            U = cfg["unroll"]
            hint = ()
            if cfg["hints"]:
                hint = (mybir.EngineType.PE, mybir.EngineType.Activation,
                        mybir.EngineType.DVE)

            def t_loop():
                nc.vector.memset(h[:], 0.0)
                if DT != f32 or cfg["asplit"]:
                    nc.vector.memset(h32[:], 0.0)
                if cfg.get("static"):
                    for i in range(0, T_steps, U):
                        for u in range(U):
                            step(i + u)
                else:
                    with tc.For_i(0, T_steps, U, hint_engines=hint,
                                  staggered_reset=cfg["staggered"]) as i:
                        for u in range(U):
                            step(i + u if u else i)

            if cfg["outer"] == 1:
                t_loop()
            else:
                with tc.For_i(0, cfg["outer"], 1):
                    t_loop()

    nc.compile()
    return nc


# --- host side -------------------------------------------------------------

def _chunk(w2d):
    """[K, M] row-chunked to [128, K//128, M]."""
    K, M = w2d.shape
    return np.ascontiguousarray(
        w2d.reshape(K // 128, 128, M).transpose(1, 0, 2))


def _wprep(w2d, np_dt_w, wsplit):
    """[K, M] -> [128, NW, K//128, M] in np_dt_w (hi/lo split if wsplit)."""
    c = _chunk(w2d.astype(np.float32))
    hi = c.astype(np_dt_w)
    if not wsplit:
        return hi[:, None]
    lo = (c - hi.astype(np.float32)).astype(np_dt_w)
    return np.ascontiguousarray(np.stack([hi, lo], axis=1))


def _prep(np_dt_w, np_dt_a, x, wb1, bb1, wb2, bb2, wff1, bff1, wff2, bff2, wta, bta, wtb, btb,
          cfg=None):
    cfg = cfg or CFG
    f32 = np.float32
    ws = cfg["wsplit"]
    w1 = _wprep(wb1.T, np_dt_w, ws)                       # [128, NW, 6, 1024]
    w2 = _wprep((1.7159 * wb2).T, np_dt_w, ws)
    wf1 = _wprep((1.7159 * wff1).T, np_dt_w, ws)
    wf2 = _wprep((1.7159 * wff2).T, np_dt_w, ws)
    wt = _wprep((1.7159 * (wta + wtb)).T, np_dt_w, ws)
    bias = np.zeros((128, 28), f32)
    bias[:, 0:8] = (0.666 * bb1).reshape(8, 128).T
    bias[:, 8:16] = (0.666 * bb2).reshape(8, 128).T
    bias[:, 16:20] = bff1.reshape(4, 128).T
    bias[:, 20:24] = bff2.reshape(4, 128).T
    bias[:, 24:28] = (0.5 if cfg['onetable'] else 1.0) * (bta + btb).reshape(4, 128).T

    wag = cfg.get("wag", False)
    if wag:
        shards = []
        for c in range(NCORES):
            parts = []
            for arr, cols in ((w1, 6 * BU), (w2, K2 * BU), (wf1, K2 * H),
                              (wf2, K2 * H), (wt, K2 * H)):
                flat = arr.reshape(128, cols)
                sl = cols // NCORES
                parts.append(flat[:, c * sl:(c + 1) * sl])
            shards.append(np.ascontiguousarray(np.concatenate(parts, axis=1)))

    in_maps = []
    for c in range(NCORES):
        xc = x[c * BC:(c + 1) * BC].astype(f32)                     # [BC, T, I]
        xTf = np.ascontiguousarray(
            xc.reshape(BC, T, KX, 128).transpose(3, 1, 2, 0))       # [128,T,KX,BC]
        hi = xTf.astype(np_dt_a)
        if cfg["asplit"]:
            lo = (xTf - hi.astype(f32)).astype(np_dt_a)
            xT = np.ascontiguousarray(
                np.stack([hi, lo], axis=3)).reshape(128, T, KX, 2 * BC)
        else:
            xT = hi
        if wag:
            in_maps.append(dict(xT=xT, wsh=shards[c], biases=bias))
        else:
            in_maps.append(dict(xT=xT, w1=w1, w2=w2, wf1=wf1, wf2=wf2, wt=wt,
                                biases=bias))
    return in_maps


_CACHE = {}
_RUNNER_CACHE = {}
LAST_EXEC_NS = None


def make_runner(nc, n_cores=NCORES):
    """jit-compiled SPMD runner with on-device zero output buffers.

    Returns (fn, in_names, out_names): fn takes per-core input dicts
    (list of n_cores dicts of np/jax arrays) and returns a list of global
    jax output arrays (concat on axis 0 across cores).
    """
    import jax
    import jax.numpy as jnp
    from jax.experimental.shard_map import shard_map
    from jax.sharding import Mesh, PartitionSpec
    from concourse.bass2jax import (_bass_exec_p, install_neuronx_cc_hook,
                                    partition_id_tensor)

    install_neuronx_cc_hook()
    partition_name = nc.partition_id_tensor.name if nc.partition_id_tensor else None
    in_names, out_names, out_avals = [], [], []
    for alloc in nc.m.functions[0].allocations:
        if not isinstance(alloc, mybir.MemoryLocationSet):
            continue
        name = alloc.memorylocations[0].name
        if alloc.kind == "ExternalInput":
            if name != partition_name:
                in_names.append(name)
        elif alloc.kind == "ExternalOutput":
            out_names.append(name)
            out_avals.append(
                (tuple(alloc.tensor_shape), mybir.dt.np(alloc.dtype)))
    all_in = list(in_names) + list(out_names)
    if partition_name is not None:
        all_in.append(partition_name)
    import jax.core as jcore
    avals = tuple(jcore.ShapedArray(s, d) for s, d in out_avals)

    def _body(*args):
        operands = list(args)
        if partition_name is not None:
            operands.append(partition_id_tensor())
        outs = _bass_exec_p.bind(
            *operands,
            out_avals=avals,
            in_names=tuple(all_in),
            out_names=tuple(out_names),
            lowering_input_output_aliases=(),
            sim_require_finite=True,
            sim_require_nnan=True,
            nc=nc,
        )
        return tuple(outs)

    devices = jax.devices()[:n_cores]
    mesh = Mesh(np.asarray(devices), ("core",))
    from jax.sharding import NamedSharding
    sharding = NamedSharding(mesh, PartitionSpec("core"))
    n_params, n_outs = len(in_names), len(out_names)
    jf = jax.jit(
        shard_map(
            _body, mesh=mesh,
            in_specs=(PartitionSpec("core"),) * (n_params + n_outs),
            out_specs=(PartitionSpec("core"),) * n_outs,
            check_rep=False),
        donate_argnums=tuple(range(n_params, n_params + n_outs)))
    # on-device zero output buffers (no host->device transfer)
    zmk = jax.jit(
        lambda: tuple(jnp.zeros((n_cores * s[0], *s[1:]), d) for s, d in out_avals),
        out_shardings=(sharding,) * n_outs)

    def fn(in_maps):
        concat_in = [
            np.concatenate([np.asarray(in_maps[c][n]) for c in range(n_cores)],
                           axis=0)
            for n in in_names
        ]
        return jf(*concat_in, *zmk())

    fn.jit = jf
    fn.make_zeros = zmk
    fn.sharding = sharding
    return fn, in_names, out_names


def kernel(**inputs):
    import ml_dtypes
    inputs = {k: np.asarray(v) for k, v in inputs.items()}
    # wide mode folds biases away; only valid when all biases are zero
    if (CFG["wide"] or CFG["preu"]) and any(
            np.any(np.asarray(inputs[k]) != 0)
            for k in ("bb1", "bb2", "bff1", "bff2", "bta", "btb")):
        CFG["wide"] = False
        CFG["preu"] = False

    def npdt(s):
        return {"float32": np.float32, "bfloat16": ml_dtypes.bfloat16}[s]

    key = tuple(sorted(CFG.items()))
    if key not in _CACHE:
        _CACHE[key] = build(T, CFG)
        _RUNNER_CACHE[key] = make_runner(_CACHE[key])
    fn, in_names, out_names = _RUNNER_CACHE[key]
    in_maps = _prep(npdt(CFG["dtype_w"]), npdt(CFG["dtype_a"]), cfg=CFG, **inputs)
    outs = fn(in_maps)
    ys_g = np.asarray(outs[out_names.index("ys")])   # [NC*T, 128, KH*BC]
    ys_g = ys_g.reshape(NCORES, T, 128, KH, BC).astype(np.float32)
    out = np.ascontiguousarray(
        ys_g.transpose(0, 4, 1, 3, 2).reshape(B, T, H))
    return out



# revision 25
# speedup vs baseline: 1.0483x; 1.0483x over previous
"""CfC RNN kernel for Trainium2 (8 NeuronCores, batch-data-parallel).

Model (per step, reference semantics, ts = 1.0):
    z_in = concat([x_t, h])                      # [B, I+H] = [B, 768]
    z1 = 1.7159*tanh(0.666*(z_in @ wb1.T + bb1)) # [B, 1024]
    z2 = 1.7159*tanh(0.666*(z1 @ wb2.T + bb2))   # [B, 1024]
    ff1 = tanh(z2 @ wff1.T + bff1)               # [B, 512]
    ff2 = tanh(z2 @ wff2.T + bff2)
    t   = sigmoid(z2 @ (wta+wtb).T + (bta+btb))  # ta*1+tb folded
    h'  = ff1 + t*(ff2-ff1)

Device layout is dim-major everywhere: [dim -> 128 partitions, batch -> free].
Algebraic folds (host-side):
  - store z1' = tanh(0.666*pre1)  (the 1.7159 is folded into wb2)
  - store z2' = tanh(0.666*pre2)  (the 1.7159 is folded into the head weights)
  - t_a*ts + t_b with ts=1 == one matmul with (wta+wtb), bias (bta+btb)
"""

import sys

sys.path.insert(0, "/opt/trn_rl_repo")

import numpy as np

import concourse.bass as bass
import concourse.tile as tile
from concourse import bacc, mybir
from concourse import bass_utils
from concourse.bass import ds, ts

B, T, I, H, BU = 64, 512, 256, 512, 1024
NCORES = 8
BC = B // NCORES  # batch rows per core
KX = I // 128     # 2  x K-chunks
KH = H // 128     # 4  h K-chunks
M1 = BU // 128    # 8  mm1 out tiles
K2 = BU // 128    # 8  mm2 K-chunks
M2 = BU // 128    # 8  mm2 out tiles
MH = H // 128     # 4  head out tiles

AF = mybir.ActivationFunctionType

# --- build configuration ---------------------------------------------------
CFG = dict(
    dtype_w="bfloat16",  # weights dtype (stationary operand)
    dtype_a="bfloat16",  # activations/x/h dtype (moving operand)
    unroll=2,           # steps per For_i iteration
    hints=False,        # hint_engines on the loop back-edge (CRASHES device!)
    staggered=True,     # staggered_reset loop semaphore recycling
    # timing-knockout flags (break correctness; for diagnosis only)
    ko_dma=False,       # drop the per-step ys DMA
    ko_dyn=False,       # static x index instead of ds(t)
    ko_act=False,       # skip ACT + DVE (PE only)
    ko_mm=False,        # skip matmuls (ACT/DVE only)
    ko_all=False,       # nearly-empty loop body
    onetable=True,      # express sigmoid via tanh => single ACT table set
    outer=1,            # whole-kernel repetitions (timing amplifier)
    wide=True,          # single wide ACT/DVE per phase (requires zero biases)
    preu=True,          # device-precompute u = 0.666*(x @ w1x.T); bf16 only
    wsplit=False,       # hi/lo split weights: W = bf16(W) + bf16(W - bf16(W))
    asplit=False,       # hi/lo split activations (z1/z2/h/x); wide mode only
    ldwopt=False,       # pass --enable-ldw-opt=true to walrus (dedups LDWEIGHTS)
    abufs=2,            # acts tile-pool buffers
    pbufs=2,            # psum tile-pool buffers
    static=False,       # python-unrolled T loop (for TimelineSim / small T)
    wag=True,           # upload 1/8 of weights per core + on-device AllGather
    ys16=True,          # ys output in bf16 (DMA the bf16 h tile directly)
    chain=True,         # chain-latency opts: u-add via identity matmul into
                        # PSUM (PE starts step early), tt-first head order,
                        # DVE h cast instead of ACT
)

# flat per-tensor weight column counts (x128 partitions, bf16)
_WCOLS = [("w1", 6 * BU), ("w2", K2 * BU), ("wf1", K2 * H), ("wf2", K2 * H),
          ("wt", K2 * H)]
WTOT = sum(c for _, c in _WCOLS)          # 26624
WSH = WTOT // NCORES                      # 3328 cols per core shard


_LDWOPT_PATCHED = False


def _patch_ldwopt():
    global _LDWOPT_PATCHED
    if _LDWOPT_PATCHED:
        return
    _LDWOPT_PATCHED = True
    orig = bass_utils.run_command

    def patched(cmd, *a, **kw):
        if isinstance(cmd, list):
            cmd = ["--enable-ldw-opt=true" if c == "--enable-ldw-opt=false" else c
                   for c in cmd]
        return orig(cmd, *a, **kw)

    bass_utils.run_command = patched


def _dt(name):
    return {"float32": mybir.dt.float32, "bfloat16": mybir.dt.bfloat16,
            "float32r": mybir.dt.float32r}[name]


def build(T_steps=T, cfg=CFG):
    DTW = _dt(cfg["dtype_w"])
    DT = _dt(cfg["dtype_a"])
    nc = bacc.Bacc("TRN2", target_bir_lowering=False, debug=False,
                   num_devices=NCORES)

    f32 = mybir.dt.float32
    NA = 2 if cfg["asplit"] else 1
    xT_d = nc.dram_tensor("xT", [128, T, KX, NA * BC], DT, kind="ExternalInput").ap()
    NW = 2 if cfg["wsplit"] else 1
    chain = (cfg.get("chain", False) and cfg["preu"] and cfg["wide"]
             and not cfg["asplit"] and not cfg["wsplit"])
    if chain:
        id_d = nc.dram_tensor("id128", [128, 128], DTW, kind="ExternalInput").ap()
    wag = cfg.get("wag", False)
    if wag:
        assert NW == 1
        wsh_d = nc.dram_tensor("wsh", [128, WSH], DTW, kind="ExternalInput").ap()
    else:
        w1_d = nc.dram_tensor("w1", [128, NW, KX + KH, BU], DTW, kind="ExternalInput").ap()
        w2_d = nc.dram_tensor("w2", [128, NW, K2, BU], DTW, kind="ExternalInput").ap()
        wf1_d = nc.dram_tensor("wf1", [128, NW, K2, H], DTW, kind="ExternalInput").ap()
        wf2_d = nc.dram_tensor("wf2", [128, NW, K2, H], DTW, kind="ExternalInput").ap()
        wt_d = nc.dram_tensor("wt", [128, NW, K2, H], DTW, kind="ExternalInput").ap()
    bias_d = nc.dram_tensor("biases", [128, 28], f32, kind="ExternalInput").ap()
    ys16 = cfg.get("ys16", False)
    if ys16:
        assert NA == 1 and DT != f32
        ys_d = nc.dram_tensor("ys", [T, 128, KH * BC], DT, kind="ExternalOutput").ap()
    else:
        ys_d = nc.dram_tensor("ys", [T, 128, KH * BC], f32, kind="ExternalOutput").ap()

    with tile.TileContext(nc) as tc:
        with tc.tile_pool(name="weights", bufs=1) as wp, \
             tc.tile_pool(name="state", bufs=1) as sp, \
             tc.tile_pool(name="acts", bufs=cfg["abufs"]) as ap_, \
             tc.tile_pool(name="psum", bufs=cfg["pbufs"], space="PSUM") as pp:
            xT = wp.tile([128, T, KX, NA * BC], DT)
            w1 = wp.tile([128, NW, KX + KH, BU], DTW)
            w2 = wp.tile([128, NW, K2, BU], DTW)
            wf1 = wp.tile([128, NW, K2, H], DTW)
            wf2 = wp.tile([128, NW, K2, H], DTW)
            wt = wp.tile([128, NW, K2, H], DTW)
            bia = wp.tile([128, 28], f32)
            nc.sync.dma_start(xT[:], xT_d[:])
            nc.sync.dma_start(bia[:], bias_d[:])
            if chain:
                id128 = wp.tile([128, 128], DTW)
                nc.sync.dma_start(id128[:], id_d[:])
            if wag:
                with tc.tile_pool(name="wag_dram", bufs=1, space="DRAM") as dram:
                    wag_in = dram.tile([128, WSH], DTW)
                    wag_out = dram.tile([NCORES * 128, WSH], DTW)
                    nc.gpsimd.dma_start(wag_in[:], wsh_d[:])
                    nc.gpsimd.collective_compute(
                        "AllGather", mybir.AluOpType.bypass,
                        replica_groups=[list(range(NCORES))],
                        ins=[wag_in.opt()], outs=[wag_out.opt()])
                    wag3 = wag_out.rearrange("(c p) s -> c p s", c=NCORES)
                    flats = {
                        "w1": w1.rearrange("p a k m -> p (a k m)"),
                        "w2": w2.rearrange("p a k m -> p (a k m)"),
                        "wf1": wf1.rearrange("p a k m -> p (a k m)"),
                        "wf2": wf2.rearrange("p a k m -> p (a k m)"),
                        "wt": wt.rearrange("p a k m -> p (a k m)"),
                    }
                    for c in range(NCORES):
                        off = 0
                        for nm, cols in _WCOLS:
                            sl = cols // NCORES
                            nc.sync.dma_start(
                                flats[nm][:, c * sl:(c + 1) * sl],
                                wag3[c, :, off:off + sl])
                            off += sl
            else:
                for sb_t, dr in ((w1, w1_d), (w2, w2_d), (wf1, wf1_d),
                                 (wf2, wf2_d), (wt, wt_d)):
                    nc.sync.dma_start(sb_t[:], dr[:])

            h = sp.tile([128, KH, NA * BC], DT)  # recurrent state, dim-major
            h32 = sp.tile([128, KH * BC], f32)   # fp32 copy for output DMA
            if DT == f32 and not cfg["asplit"]:
                h = h32.rearrange("p (c b) -> p c b", c=KH)

            u_sb = None
            if cfg["preu"]:
                u_sb = wp.tile([128, T_steps, M1 * BC], DT)
                TB = 512 // BC                   # time-block per psum bank
                for m in range(M1):
                    for nb in range((T_steps + TB - 1) // TB):
                        nsz = min(TB, T_steps - nb * TB)
                        # tag shared with the steady-state hps tile: preu only
                        # runs in the prologue, and 4 tags x 2 bufs = 8 banks
                        pu = pp.tile([128, TB * BC], f32,
                                     tag="hps" if chain else "pu")
                        for p in range(NW):
                            for k in range(KX):
                                for a in range(NA):
                                    nc.tensor.matmul(
                                        pu[:, 0:nsz * BC],
                                        w1[:, p, k, ts(m, 128)],
                                        xT[:, nb * TB:nb * TB + nsz, k, ts(a, BC)],
                                        start=(p == 0 and k == 0 and a == 0),
                                        stop=(p == NW - 1 and k == KX - 1
                                              and a == NA - 1))
                        nc.scalar.activation(
                            u_sb[:, nb * TB:nb * TB + nsz, ts(m, BC)],
                            pu[:, 0:nsz * BC], AF.Copy, bias=0.0,
                            scale=1.0 if chain else 0.666)

            def step(t_idx):
                if cfg["ko_all"]:
                    d0 = ap_.tile([128, BC], f32, tag="d0")
                    nc.vector.memset(d0[:], 0.0)
                    return
                x_idx = ds(t_idx, 1) if not cfg["ko_dyn"] else ds(0, 1)
                if not cfg["ko_act"]:
                    z1 = ap_.tile([128, M1, NA * BC], DT, tag="z1")
                    z2 = ap_.tile([128, M2, NA * BC], DT, tag="z2")
                    if cfg["asplit"]:
                        z1f = ap_.tile([128, M1 * BC], f32, tag="z1f")
                        z2f = ap_.tile([128, M2 * BC], f32, tag="z2f")
                    ff1 = ap_.tile([128, MH * BC], f32, tag="ff1")
                    ff2 = ap_.tile([128, MH * BC], f32, tag="ff2")
                    tt = ap_.tile([128, MH * BC], f32, tag="tt")
                else:
                    z1 = z2 = h
                    ff1 = ff2 = tt = h
                if not cfg["ko_mm"]:
                    z1ps = pp.tile([128, M1 * BC], f32, tag="z1ps")
                    z2ps = pp.tile([128, M2 * BC], f32, tag="z2ps")
                    if chain:
                        # tt in its own bank so ACT(tt) overlaps ff1/ff2 MMs
                        hps = pp.tile([128, 2 * MH * BC], f32, tag="hps")
                        tps = pp.tile([128, MH * BC], f32, tag="tps")
                    else:
                        hps = pp.tile([128, 3 * MH * BC], f32, tag="hps")

                # ---- mm1: z1pre = [x_t; h] @ wb1.T  (K = 2 x-chunks + 4 h-chunks)
                if chain and not cfg["ko_mm"]:
                    # u(t) seeded into PSUM via identity matmul: no h
                    # dependency, so PE starts the step during the previous
                    # step's tail; the DVE zpre stage disappears.
                    nc.tensor.matmul(z1ps[:], id128[:, :],
                                     u_sb[:, x_idx, :], start=True, stop=False)
                    for m in range(M1):
                        for k in range(KH):
                            nc.tensor.matmul(
                                z1ps[:, ts(m, BC)],
                                w1[:, 0, KX + k, ts(m, 128)],
                                h[:, k, 0:BC],
                                start=False,
                                stop=(m == M1 - 1 and k == KH - 1))
                else:
                    for m in range(M1 if not cfg["ko_mm"] else 0):
                        first = True
                        for p in range(NW):
                            if not cfg["preu"]:
                                for k in range(KX):
                                    for a in range(NA):
                                        nc.tensor.matmul(
                                            z1ps[:, ts(m, BC)],
                                            w1[:, p, k, ts(m, 128)],
                                            xT[:, x_idx, k, ts(a, BC)],
                                            start=first, stop=False)
                                        first = False
                            for k in range(KH):
                                for a in range(NA):
                                    nc.tensor.matmul(
                                        z1ps[:, ts(m, BC)],
                                        w1[:, p, KX + k, ts(m, 128)],
                                        h[:, k, ts(a, BC)],
                                        start=first,
                                        stop=(p == NW - 1 and k == KH - 1
                                              and a == NA - 1))
                                    first = False
                # z1 = tanh(0.666*pre + 0.666*bb1)
                def split_phase(dst, dstf, src_ps, scale):
                    # dst: [128, M, 2*BC] hi/lo bf16; dstf: [128, M*BC] f32
                    M = dst.shape[1]
                    dst3 = dst.rearrange("p m (a b) -> p m a b", a=NA)
                    nc.scalar.activation(dstf[:], src_ps[:], AF.Tanh, scale=scale)
                    dfv = dstf.rearrange("p (m b) -> p m b", m=M)
                    nc.scalar.activation(dst3[:, :, 0, :], dfv[:], AF.Copy)
                    nc.vector.tensor_sub(dst3[:, :, 1, :], dfv[:], dst3[:, :, 0, :])

                if chain and not cfg["ko_act"]:
                    nc.scalar.activation(z1.rearrange("p m b -> p (m b)"),
                                         z1ps[:], AF.Tanh, scale=0.666)
                elif cfg["preu"] and not cfg["ko_act"]:
                    zpre = ap_.tile([128, M1 * BC], f32, tag="zpre")
                    nc.vector.scalar_tensor_tensor(
                        zpre[:], z1ps[:], 0.666, u_sb[:, ds(t_idx, 1), :],
                        mybir.AluOpType.mult, mybir.AluOpType.add)
                    if cfg["asplit"]:
                        split_phase(z1, z1f, zpre, 1.0)
                    else:
                        nc.scalar.activation(z1.rearrange("p m b -> p (m b)"),
                                             zpre[:], AF.Tanh)
                elif cfg["wide"] and not cfg["ko_act"]:
                    if cfg["asplit"]:
                        split_phase(z1, z1f, z1ps, 0.666)
                    else:
                        nc.scalar.activation(z1.rearrange("p m b -> p (m b)"),
                                             z1ps[:], AF.Tanh, scale=0.666)
                else:
                    for m in range(M1 if not cfg["ko_act"] else 0):
                        nc.scalar.activation(z1[:, m, :], z1ps[:, ts(m, BC)],
                                             AF.Tanh, bias=bia[:, m:m + 1], scale=0.666)

                # ---- mm2: z2pre = z1 @ (1.7159*wb2).T
                for m in range(M2 if not cfg["ko_mm"] else 0):
                    for p in range(NW):
                        for k in range(K2):
                            for a in range(NA):
                                nc.tensor.matmul(
                                    z2ps[:, ts(m, BC)],
                                    w2[:, p, k, ts(m, 128)],
                                    z1[:, k if not cfg["ko_act"] else k % KH,
                                       ts(a, BC)],
                                    start=(p == 0 and k == 0 and a == 0),
                                    stop=(p == NW - 1 and k == K2 - 1
                                          and a == NA - 1))
                if cfg["wide"] and not cfg["ko_act"]:
                    if cfg["asplit"]:
                        split_phase(z2, z2f, z2ps, 0.666)
                    else:
                        nc.scalar.activation(z2.rearrange("p m b -> p (m b)"),
                                             z2ps[:], AF.Tanh, scale=0.666)
                else:
                    for m in range(M2 if not cfg["ko_act"] else 0):
                        nc.scalar.activation(z2[:, m, :], z2ps[:, ts(m, BC)],
                                             AF.Tanh, bias=bia[:, 8 + m:9 + m], scale=0.666)

                # ---- heads: ff1, ff2, t (weights pre-scaled by 1.7159)
                # chain: emit tt first so its ACT overlaps the ff1/ff2 matmuls
                # and the DVE tail starts right after ff2's ACT.
                head_seq = ((2, wt), (0, wf1), (1, wf2)) if chain else \
                           ((0, wf1), (1, wf2), (2, wt))
                for hd, w_sb in (head_seq if not cfg["ko_mm"] else ()):
                    for m in range(MH):
                        for p in range(NW):
                            for k in range(K2):
                                for a in range(NA):
                                    dst = (tps[:, ts(m, BC)]
                                           if (chain and hd == 2) else
                                           hps[:, ts(hd * MH + m, BC)])
                                    nc.tensor.matmul(
                                        dst,
                                        w_sb[:, p, k, ts(m, 128)],
                                        z2[:, k if not cfg["ko_act"] else k % KH,
                                           ts(a, BC)],
                                        start=(p == 0 and k == 0 and a == 0),
                                        stop=(p == NW - 1 and k == K2 - 1
                                              and a == NA - 1))
                if cfg["wide"] and not cfg["ko_act"]:
                    if chain:
                        nc.scalar.activation(tt[:], tps[:], AF.Tanh, scale=0.5)
                        nc.scalar.activation(ff1[:], hps[:, 0:MH * BC], AF.Tanh)
                        nc.scalar.activation(ff2[:], hps[:, MH * BC:2 * MH * BC],
                                             AF.Tanh)
                    else:
                        # ff1|ff2 in one tanh over hps cols [0, 2*MH*BC)
                        nc.scalar.activation(ff1[:], hps[:, 0:MH * BC], AF.Tanh)
                        nc.scalar.activation(ff2[:], hps[:, MH * BC:2 * MH * BC],
                                             AF.Tanh)
                        nc.scalar.activation(tt[:], hps[:, 2 * MH * BC:3 * MH * BC],
                                             AF.Tanh, scale=0.5)
                else:
                    for m in range(MH if not cfg["ko_act"] else 0):
                        nc.scalar.activation(ff1[:, ts(m, BC)], hps[:, ts(m, BC)],
                                             AF.Tanh, bias=bia[:, 16 + m:17 + m])
                    for m in range(MH if not cfg["ko_act"] else 0):
                        nc.scalar.activation(ff2[:, ts(m, BC)], hps[:, ts(MH + m, BC)],
                                             AF.Tanh, bias=bia[:, 20 + m:21 + m])
                    for m in range(MH if not cfg["ko_act"] else 0):
                        if cfg["onetable"]:
                            nc.scalar.activation(tt[:, ts(m, BC)], hps[:, ts(2 * MH + m, BC)],
                                                 AF.Tanh, bias=bia[:, 24 + m:25 + m],
                                                 scale=0.5)
                        else:
                            nc.scalar.activation(tt[:, ts(m, BC)], hps[:, ts(2 * MH + m, BC)],
                                                 AF.Sigmoid, bias=bia[:, 24 + m:25 + m])

                # ---- h' = ff1 + t*(ff2-ff1); onetable: t = 0.5*(1+tt_raw)
                if cfg["wide"] and not cfg["ko_act"]:
                    W = MH * BC
                    d = ap_.tile([128, W], f32, tag="d")
                    e = ap_.tile([128, W], f32, tag="e")
                    # e first: depends only on tt, overlaps the ff1/ff2 ACTs
                    nc.vector.tensor_scalar_add(e[:], tt[:], 1.0)
                    nc.vector.tensor_sub(d[:], ff2[:], ff1[:])
                    nc.vector.tensor_mul(d[:], d[:], e[:])
                    nc.vector.scalar_tensor_tensor(
                        h32[:], d[:], 0.5, ff1[:],
                        mybir.AluOpType.mult, mybir.AluOpType.add)
                for c in range(0 if (cfg["wide"] or cfg["ko_act"]) else KH):
                    d = ap_.tile([128, BC], f32, tag="d")
                    e = ap_.tile([128, BC], f32, tag="e")
                    nc.vector.tensor_sub(d[:], ff2[:, ts(c, BC)], ff1[:, ts(c, BC)])
                    if cfg["onetable"]:
                        nc.vector.tensor_scalar_add(e[:], tt[:, ts(c, BC)], 1.0)
                        nc.vector.tensor_mul(d[:], d[:], e[:])
                        nc.vector.scalar_tensor_tensor(
                            h32[:, ts(c, BC)], d[:], 0.5, ff1[:, ts(c, BC)],
                            mybir.AluOpType.mult, mybir.AluOpType.add)
                    else:
                        nc.vector.tensor_mul(e[:], d[:], tt[:, ts(c, BC)])
                        nc.vector.tensor_add(h32[:, ts(c, BC)], e[:], ff1[:, ts(c, BC)])
                if not cfg["ko_act"] and (DT != f32 or cfg["asplit"]):
                    h3 = h.rearrange("p c (a b) -> p c a b", a=NA)
                    h32v = h32.rearrange("p (c b) -> p c b", c=KH)
                    if chain:
                        # DVE copy keeps the tail on one engine (ACT hop saved)
                        nc.vector.tensor_copy(h3[:, :, 0, :], h32v[:])
                    else:
                        nc.scalar.activation(h3[:, :, 0, :], h32v[:], AF.Copy)
                    if cfg["asplit"]:
                        nc.vector.tensor_sub(h3[:, :, 1, :], h32v[:], h3[:, :, 0, :])

                if not cfg["ko_dma"]:
                    if ys16:
                        nc.sync.dma_start(ys_d[ds(t_idx, 1), :, :],
                                          h.rearrange("p c b -> p (c b)")[:])
                    else:
                        nc.sync.dma_start(ys_d[ds(t_idx, 1), :, :], h32[:])

            U = cfg["unroll"]
            hint = ()
            if cfg["hints"]:
                hint = (mybir.EngineType.PE, mybir.EngineType.Activation,
                        mybir.EngineType.DVE)

            def t_loop():
                nc.vector.memset(h[:], 0.0)
                if DT != f32 or cfg["asplit"]:
                    nc.vector.memset(h32[:], 0.0)
                if cfg.get("static"):
                    for i in range(0, T_steps, U):
                        for u in range(U):
                            step(i + u)
                else:
                    with tc.For_i(0, T_steps, U, hint_engines=hint,
                                  staggered_reset=cfg["staggered"]) as i:
                        for u in range(U):
                            step(i + u if u else i)

            if cfg["outer"] == 1:
                t_loop()
            else:
                with tc.For_i(0, cfg["outer"], 1):
                    t_loop()

    nc.compile()
    return nc


# --- host side -------------------------------------------------------------

def _chunk(w2d):
    """[K, M] row-chunked to [128, K//128, M]."""
    K, M = w2d.shape
    return np.ascontiguousarray(
        w2d.reshape(K // 128, 128, M).transpose(1, 0, 2))


def _wprep(w2d, np_dt_w, wsplit):
    """[K, M] -> [128, NW, K//128, M] in np_dt_w (hi/lo split if wsplit)."""
    c = _chunk(w2d.astype(np.float32))
    hi = c.astype(np_dt_w)
    if not wsplit:
        return hi[:, None]
    lo = (c - hi.astype(np.float32)).astype(np_dt_w)
    return np.ascontiguousarray(np.stack([hi, lo], axis=1))


def _prep(np_dt_w, np_dt_a, x, wb1, bb1, wb2, bb2, wff1, bff1, wff2, bff2, wta, bta, wtb, btb,
          cfg=None):
    cfg = cfg or CFG
    f32 = np.float32
    ws = cfg["wsplit"]
    w1 = _wprep(wb1.T, np_dt_w, ws)                       # [128, NW, 6, 1024]
    w2 = _wprep((1.7159 * wb2).T, np_dt_w, ws)
    wf1 = _wprep((1.7159 * wff1).T, np_dt_w, ws)
    wf2 = _wprep((1.7159 * wff2).T, np_dt_w, ws)
    wt = _wprep((1.7159 * (wta + wtb)).T, np_dt_w, ws)
    bias = np.zeros((128, 28), f32)
    bias[:, 0:8] = (0.666 * bb1).reshape(8, 128).T
    bias[:, 8:16] = (0.666 * bb2).reshape(8, 128).T
    bias[:, 16:20] = bff1.reshape(4, 128).T
    bias[:, 20:24] = bff2.reshape(4, 128).T
    bias[:, 24:28] = (0.5 if cfg['onetable'] else 1.0) * (bta + btb).reshape(4, 128).T

    wag = cfg.get("wag", False)
    if wag:
        shards = []
        for c in range(NCORES):
            parts = []
            for arr, cols in ((w1, 6 * BU), (w2, K2 * BU), (wf1, K2 * H),
                              (wf2, K2 * H), (wt, K2 * H)):
                flat = arr.reshape(128, cols)
                sl = cols // NCORES
                parts.append(flat[:, c * sl:(c + 1) * sl])
            shards.append(np.ascontiguousarray(np.concatenate(parts, axis=1)))

    in_maps = []
    for c in range(NCORES):
        xc = x[c * BC:(c + 1) * BC].astype(f32)                     # [BC, T, I]
        xTf = np.ascontiguousarray(
            xc.reshape(BC, T, KX, 128).transpose(3, 1, 2, 0))       # [128,T,KX,BC]
        hi = xTf.astype(np_dt_a)
        if cfg["asplit"]:
            lo = (xTf - hi.astype(f32)).astype(np_dt_a)
            xT = np.ascontiguousarray(
                np.stack([hi, lo], axis=3)).reshape(128, T, KX, 2 * BC)
        else:
            xT = hi
        if wag:
            m = dict(xT=xT, wsh=shards[c], biases=bias)
        else:
            m = dict(xT=xT, w1=w1, w2=w2, wf1=wf1, wf2=wf2, wt=wt, biases=bias)
        if (cfg.get("chain", False) and cfg["preu"] and cfg["wide"]
                and not cfg["asplit"] and not cfg["wsplit"]):
            m["id128"] = np.eye(128, dtype=np_dt_w)
        in_maps.append(m)
    return in_maps


_CACHE = {}
_RUNNER_CACHE = {}
LAST_EXEC_NS = None


def make_runner(nc, n_cores=NCORES):
    """jit-compiled SPMD runner with on-device zero output buffers.

    Returns (fn, in_names, out_names): fn takes per-core input dicts
    (list of n_cores dicts of np/jax arrays) and returns a list of global
    jax output arrays (concat on axis 0 across cores).
    """
    import jax
    import jax.numpy as jnp
    from jax.experimental.shard_map import shard_map
    from jax.sharding import Mesh, PartitionSpec
    from concourse.bass2jax import (_bass_exec_p, install_neuronx_cc_hook,
                                    partition_id_tensor)

    install_neuronx_cc_hook()
    partition_name = nc.partition_id_tensor.name if nc.partition_id_tensor else None
    in_names, out_names, out_avals = [], [], []
    for alloc in nc.m.functions[0].allocations:
        if not isinstance(alloc, mybir.MemoryLocationSet):
            continue
        name = alloc.memorylocations[0].name
        if alloc.kind == "ExternalInput":
            if name != partition_name:
                in_names.append(name)
        elif alloc.kind == "ExternalOutput":
            out_names.append(name)
            out_avals.append(
                (tuple(alloc.tensor_shape), mybir.dt.np(alloc.dtype)))
    all_in = list(in_names) + list(out_names)
    if partition_name is not None:
        all_in.append(partition_name)
    import jax.core as jcore
    avals = tuple(jcore.ShapedArray(s, d) for s, d in out_avals)

    def _body(*args):
        operands = list(args)
        if partition_name is not None:
            operands.append(partition_id_tensor())
        outs = _bass_exec_p.bind(
            *operands,
            out_avals=avals,
            in_names=tuple(all_in),
            out_names=tuple(out_names),
            lowering_input_output_aliases=(),
            sim_require_finite=True,
            sim_require_nnan=True,
            nc=nc,
        )
        return tuple(outs)

    devices = jax.devices()[:n_cores]
    mesh = Mesh(np.asarray(devices), ("core",))
    from jax.sharding import NamedSharding
    sharding = NamedSharding(mesh, PartitionSpec("core"))
    n_params, n_outs = len(in_names), len(out_names)
    jf = jax.jit(
        shard_map(
            _body, mesh=mesh,
            in_specs=(PartitionSpec("core"),) * (n_params + n_outs),
            out_specs=(PartitionSpec("core"),) * n_outs,
            check_rep=False),
        donate_argnums=tuple(range(n_params, n_params + n_outs)))
    # on-device zero output buffers (no host->device transfer)
    zmk = jax.jit(
        lambda: tuple(jnp.zeros((n_cores * s[0], *s[1:]), d) for s, d in out_avals),
        out_shardings=(sharding,) * n_outs)

    def fn(in_maps):
        concat_in = [
            np.concatenate([np.asarray(in_maps[c][n]) for c in range(n_cores)],
                           axis=0)
            for n in in_names
        ]
        return jf(*concat_in, *zmk())

    fn.jit = jf
    fn.make_zeros = zmk
    fn.sharding = sharding
    return fn, in_names, out_names


def kernel(**inputs):
    import ml_dtypes
    inputs = {k: np.asarray(v) for k, v in inputs.items()}
    # wide mode folds biases away; only valid when all biases are zero
    if (CFG["wide"] or CFG["preu"]) and any(
            np.any(np.asarray(inputs[k]) != 0)
            for k in ("bb1", "bb2", "bff1", "bff2", "bta", "btb")):
        CFG["wide"] = False
        CFG["preu"] = False

    def npdt(s):
        return {"float32": np.float32, "bfloat16": ml_dtypes.bfloat16}[s]

    key = tuple(sorted(CFG.items()))
    if key not in _CACHE:
        _CACHE[key] = build(T, CFG)
        _RUNNER_CACHE[key] = make_runner(_CACHE[key])
    fn, in_names, out_names = _RUNNER_CACHE[key]
    in_maps = _prep(npdt(CFG["dtype_w"]), npdt(CFG["dtype_a"]), cfg=CFG, **inputs)
    outs = fn(in_maps)
    ys_g = np.asarray(outs[out_names.index("ys")])   # [NC*T, 128, KH*BC]
    ys_g = ys_g.reshape(NCORES, T, 128, KH, BC).astype(np.float32)
    out = np.ascontiguousarray(
        ys_g.transpose(0, 4, 1, 3, 2).reshape(B, T, H))
    return out



# revision 36
# speedup vs baseline: 1.2408x; 1.1836x over previous
"""CfC RNN kernel for Trainium2 (8 NeuronCores, batch-data-parallel).

Model (per step, reference semantics, ts = 1.0):
    z_in = concat([x_t, h])                      # [B, I+H] = [B, 768]
    z1 = 1.7159*tanh(0.666*(z_in @ wb1.T + bb1)) # [B, 1024]
    z2 = 1.7159*tanh(0.666*(z1 @ wb2.T + bb2))   # [B, 1024]
    ff1 = tanh(z2 @ wff1.T + bff1)               # [B, 512]
    ff2 = tanh(z2 @ wff2.T + bff2)
    t   = sigmoid(z2 @ (wta+wtb).T + (bta+btb))  # ta*1+tb folded
    h'  = ff1 + t*(ff2-ff1)

Device layout is dim-major everywhere: [dim -> 128 partitions, batch -> free].
Algebraic folds (host-side):
  - store z1' = tanh(0.666*pre1)  (the 1.7159 is folded into wb2)
  - store z2' = tanh(0.666*pre2)  (the 1.7159 is folded into the head weights)
  - t_a*ts + t_b with ts=1 == one matmul with (wta+wtb), bias (bta+btb)
"""

import sys

sys.path.insert(0, "/opt/trn_rl_repo")

import numpy as np

import concourse.bass as bass
import concourse.tile as tile
from concourse import bacc, mybir
from concourse import bass_utils
from concourse.bass import ds, ts

B, T, I, H, BU = 64, 512, 256, 512, 1024
NCORES = 8
BC = B // NCORES  # batch rows per core
KX = I // 128     # 2  x K-chunks
KH = H // 128     # 4  h K-chunks
M1 = BU // 128    # 8  mm1 out tiles
K2 = BU // 128    # 8  mm2 K-chunks
M2 = BU // 128    # 8  mm2 out tiles
MH = H // 128     # 4  head out tiles

AF = mybir.ActivationFunctionType

# --- build configuration ---------------------------------------------------
CFG = dict(
    dtype_w="bfloat16",  # weights dtype (stationary operand)
    dtype_a="bfloat16",  # activations/x/h dtype (moving operand)
    unroll=8,           # steps per For_i iteration
    hints=False,        # hint_engines on the loop back-edge (CRASHES device!)
    staggered=True,     # staggered_reset loop semaphore recycling
    # timing-knockout flags (break correctness; for diagnosis only)
    ko_dma=False,       # drop the per-step ys DMA
    ko_dyn=False,       # static x index instead of ds(t)
    ko_act=False,       # skip ACT + DVE (PE only)
    ko_mm=False,        # skip matmuls (ACT/DVE only)
    ko_all=False,       # nearly-empty loop body
    onetable=True,      # express sigmoid via tanh => single ACT table set
    outer=1,            # whole-kernel repetitions (timing amplifier)
    wide=True,          # single wide ACT/DVE per phase (requires zero biases)
    preu=True,          # device-precompute u = 0.666*(x @ w1x.T); bf16 only
    wsplit=False,       # hi/lo split weights: W = bf16(W) + bf16(W - bf16(W))
    asplit=False,       # hi/lo split activations (z1/z2/h/x); wide mode only
    ldwopt=False,       # pass --enable-ldw-opt=true to walrus (dedups LDWEIGHTS)
    abufs=2,            # acts tile-pool buffers
    pbufs=2,            # psum tile-pool buffers
    static=False,       # python-unrolled T loop (for TimelineSim / small T)
    wag=True,           # upload 1/8 of weights per core + on-device AllGather
    ys16=True,          # ys output in bf16 (DMA the bf16 h tile directly)
    chain=True,         # chain-latency opts: u-add via identity matmul into
                        # PSUM (PE starts step early), tt-first head order,
                        # DVE h cast instead of ACT
    chain2=True,        # split z1ps/z2ps into A/B banks so each half's tanh
                        # overlaps the other half's matmuls; single ff1|ff2 ACT
)

# flat per-tensor weight column counts (x128 partitions, bf16)
_WCOLS = [("w1", 6 * BU), ("w2", K2 * BU), ("wf1", K2 * H), ("wf2", K2 * H),
          ("wt", K2 * H)]
WTOT = sum(c for _, c in _WCOLS)          # 26624
WSH = WTOT // NCORES                      # 3328 cols per core shard


_LDWOPT_PATCHED = False


def _patch_ldwopt():
    global _LDWOPT_PATCHED
    if _LDWOPT_PATCHED:
        return
    _LDWOPT_PATCHED = True
    orig = bass_utils.run_command

    def patched(cmd, *a, **kw):
        if isinstance(cmd, list):
            cmd = ["--enable-ldw-opt=true" if c == "--enable-ldw-opt=false" else c
                   for c in cmd]
        return orig(cmd, *a, **kw)

    bass_utils.run_command = patched


def _dt(name):
    return {"float32": mybir.dt.float32, "bfloat16": mybir.dt.bfloat16,
            "float32r": mybir.dt.float32r}[name]


def build(T_steps=T, cfg=CFG):
    DTW = _dt(cfg["dtype_w"])
    DT = _dt(cfg["dtype_a"])
    nc = bacc.Bacc("TRN2", target_bir_lowering=False, debug=False,
                   num_devices=NCORES)

    f32 = mybir.dt.float32
    NA = 2 if cfg["asplit"] else 1
    xT_d = nc.dram_tensor("xT", [128, T, KX, NA * BC], DT, kind="ExternalInput").ap()
    NW = 2 if cfg["wsplit"] else 1
    chain = (cfg.get("chain", False) and cfg["preu"] and cfg["wide"]
             and not cfg["asplit"] and not cfg["wsplit"])
    chain2 = chain and cfg.get("chain2", False)
    if chain:
        id_d = nc.dram_tensor("id128", [128, 128], DTW, kind="ExternalInput").ap()
    wag = cfg.get("wag", False)
    if wag:
        assert NW == 1
        wsh_d = nc.dram_tensor("wsh", [128, WSH], DTW, kind="ExternalInput").ap()
    else:
        w1_d = nc.dram_tensor("w1", [128, NW, KX + KH, BU], DTW, kind="ExternalInput").ap()
        w2_d = nc.dram_tensor("w2", [128, NW, K2, BU], DTW, kind="ExternalInput").ap()
        wf1_d = nc.dram_tensor("wf1", [128, NW, K2, H], DTW, kind="ExternalInput").ap()
        wf2_d = nc.dram_tensor("wf2", [128, NW, K2, H], DTW, kind="ExternalInput").ap()
        wt_d = nc.dram_tensor("wt", [128, NW, K2, H], DTW, kind="ExternalInput").ap()
    bias_d = nc.dram_tensor("biases", [128, 28], f32, kind="ExternalInput").ap()
    ys16 = cfg.get("ys16", False)
    if ys16:
        assert NA == 1 and DT != f32
        ys_d = nc.dram_tensor("ys", [T, 128, KH * BC], DT, kind="ExternalOutput").ap()
    else:
        ys_d = nc.dram_tensor("ys", [T, 128, KH * BC], f32, kind="ExternalOutput").ap()

    with tile.TileContext(nc) as tc:
        with tc.tile_pool(name="weights", bufs=1) as wp, \
             tc.tile_pool(name="state", bufs=1) as sp, \
             tc.tile_pool(name="acts", bufs=cfg["abufs"]) as ap_, \
             tc.tile_pool(name="psum", bufs=cfg["pbufs"], space="PSUM") as pp:
            xT = wp.tile([128, T, KX, NA * BC], DT)
            w1 = wp.tile([128, NW, KX + KH, BU], DTW)
            w2 = wp.tile([128, NW, K2, BU], DTW)
            wf1 = wp.tile([128, NW, K2, H], DTW)
            wf2 = wp.tile([128, NW, K2, H], DTW)
            wt = wp.tile([128, NW, K2, H], DTW)
            bia = wp.tile([128, 28], f32)
            nc.sync.dma_start(xT[:], xT_d[:])
            nc.sync.dma_start(bia[:], bias_d[:])
            if chain:
                id128 = wp.tile([128, 128], DTW)
                nc.sync.dma_start(id128[:], id_d[:])
            if wag:
                with tc.tile_pool(name="wag_dram", bufs=1, space="DRAM") as dram:
                    wag_in = dram.tile([128, WSH], DTW)
                    wag_out = dram.tile([NCORES * 128, WSH], DTW)
                    nc.gpsimd.dma_start(wag_in[:], wsh_d[:])
                    nc.gpsimd.collective_compute(
                        "AllGather", mybir.AluOpType.bypass,
                        replica_groups=[list(range(NCORES))],
                        ins=[wag_in.opt()], outs=[wag_out.opt()])
                    wag3 = wag_out.rearrange("(c p) s -> c p s", c=NCORES)
                    flats = {
                        "w1": w1.rearrange("p a k m -> p (a k m)"),
                        "w2": w2.rearrange("p a k m -> p (a k m)"),
                        "wf1": wf1.rearrange("p a k m -> p (a k m)"),
                        "wf2": wf2.rearrange("p a k m -> p (a k m)"),
                        "wt": wt.rearrange("p a k m -> p (a k m)"),
                    }
                    for c in range(NCORES):
                        off = 0
                        for nm, cols in _WCOLS:
                            sl = cols // NCORES
                            nc.sync.dma_start(
                                flats[nm][:, c * sl:(c + 1) * sl],
                                wag3[c, :, off:off + sl])
                            off += sl
            else:
                for sb_t, dr in ((w1, w1_d), (w2, w2_d), (wf1, wf1_d),
                                 (wf2, wf2_d), (wt, wt_d)):
                    nc.sync.dma_start(sb_t[:], dr[:])

            h = sp.tile([128, KH, NA * BC], DT)  # recurrent state, dim-major
            h32 = sp.tile([128, KH * BC], f32)   # fp32 copy for output DMA
            if DT == f32 and not cfg["asplit"]:
                h = h32.rearrange("p (c b) -> p c b", c=KH)

            u_sb = None
            if cfg["preu"]:
                u_sb = wp.tile([128, T_steps, M1 * BC], DT)
                TB = 512 // BC                   # time-block per psum bank
                for m in range(M1):
                    for nb in range((T_steps + TB - 1) // TB):
                        nsz = min(TB, T_steps - nb * TB)
                        # tag shared with the steady-state hps tile: preu only
                        # runs in the prologue, and 4 tags x 2 bufs = 8 banks
                        pu = pp.tile([128, TB * BC], f32,
                                     tag="hps" if chain else "pu")
                        for p in range(NW):
                            for k in range(KX):
                                for a in range(NA):
                                    nc.tensor.matmul(
                                        pu[:, 0:nsz * BC],
                                        w1[:, p, k, ts(m, 128)],
                                        xT[:, nb * TB:nb * TB + nsz, k, ts(a, BC)],
                                        start=(p == 0 and k == 0 and a == 0),
                                        stop=(p == NW - 1 and k == KX - 1
                                              and a == NA - 1))
                        nc.scalar.activation(
                            u_sb[:, nb * TB:nb * TB + nsz, ts(m, BC)],
                            pu[:, 0:nsz * BC], AF.Copy, bias=0.0,
                            scale=1.0 if chain else 0.666)

            def step(t_idx):
                if cfg["ko_all"]:
                    d0 = ap_.tile([128, BC], f32, tag="d0")
                    nc.vector.memset(d0[:], 0.0)
                    return
                x_idx = ds(t_idx, 1) if not cfg["ko_dyn"] else ds(0, 1)
                if not cfg["ko_act"]:
                    z1 = ap_.tile([128, M1, NA * BC], DT, tag="z1")
                    z2 = ap_.tile([128, M2, NA * BC], DT, tag="z2")
                    if cfg["asplit"]:
                        z1f = ap_.tile([128, M1 * BC], f32, tag="z1f")
                        z2f = ap_.tile([128, M2 * BC], f32, tag="z2f")
                    if chain2:
                        ff12 = ap_.tile([128, 2 * MH * BC], f32, tag="ff1")
                        ff1 = ff2 = None
                    else:
                        ff1 = ap_.tile([128, MH * BC], f32, tag="ff1")
                        ff2 = ap_.tile([128, MH * BC], f32, tag="ff2")
                    tt = ap_.tile([128, MH * BC], f32, tag="tt")
                else:
                    z1 = z2 = h
                    ff1 = ff2 = tt = h
                if not cfg["ko_mm"]:
                    if chain2:
                        H1, H2 = M1 // 2, M2 // 2
                        z1psA = pp.tile([128, H1 * BC], f32, tag="z1ps")
                        z1psB = pp.tile([128, H1 * BC], f32, tag="z1ps")
                        z2psA = pp.tile([128, H2 * BC], f32, tag="z2ps")
                        z2psB = pp.tile([128, H2 * BC], f32, tag="z2ps")
                    else:
                        z1ps = pp.tile([128, M1 * BC], f32, tag="z1ps")
                        z2ps = pp.tile([128, M2 * BC], f32, tag="z2ps")
                    if chain:
                        # tt in its own bank so ACT(tt) overlaps ff1/ff2 MMs
                        hps = pp.tile([128, 2 * MH * BC], f32, tag="hps")
                        tps = pp.tile([128, MH * BC], f32, tag="tps")
                    else:
                        hps = pp.tile([128, 3 * MH * BC], f32, tag="hps")

                # ---- mm1: z1pre = [x_t; h] @ wb1.T  (K = 2 x-chunks + 4 h-chunks)
                if chain2 and not cfg["ko_mm"]:
                    # A/B bank halves: tanh(A) overlaps the B-half matmuls
                    for hf, zp in ((0, z1psA), (1, z1psB)):
                        nc.tensor.matmul(
                            zp[:], id128[:, :],
                            u_sb[:, x_idx, hf * H1 * BC:(hf + 1) * H1 * BC],
                            start=True, stop=False)
                        for m in range(H1):
                            for k in range(KH):
                                nc.tensor.matmul(
                                    zp[:, ts(m, BC)],
                                    w1[:, 0, KX + k, ts(hf * H1 + m, 128)],
                                    h[:, k, 0:BC],
                                    start=False,
                                    stop=(m == H1 - 1 and k == KH - 1))
                elif chain and not cfg["ko_mm"]:
                    # u(t) seeded into PSUM via identity matmul: no h
                    # dependency, so PE starts the step during the previous
                    # step's tail; the DVE zpre stage disappears.
                    nc.tensor.matmul(z1ps[:], id128[:, :],
                                     u_sb[:, x_idx, :], start=True, stop=False)
                    for m in range(M1):
                        for k in range(KH):
                            nc.tensor.matmul(
                                z1ps[:, ts(m, BC)],
                                w1[:, 0, KX + k, ts(m, 128)],
                                h[:, k, 0:BC],
                                start=False,
                                stop=(m == M1 - 1 and k == KH - 1))
                else:
                    for m in range(M1 if not cfg["ko_mm"] else 0):
                        first = True
                        for p in range(NW):
                            if not cfg["preu"]:
                                for k in range(KX):
                                    for a in range(NA):
                                        nc.tensor.matmul(
                                            z1ps[:, ts(m, BC)],
                                            w1[:, p, k, ts(m, 128)],
                                            xT[:, x_idx, k, ts(a, BC)],
                                            start=first, stop=False)
                                        first = False
                            for k in range(KH):
                                for a in range(NA):
                                    nc.tensor.matmul(
                                        z1ps[:, ts(m, BC)],
                                        w1[:, p, KX + k, ts(m, 128)],
                                        h[:, k, ts(a, BC)],
                                        start=first,
                                        stop=(p == NW - 1 and k == KH - 1
                                              and a == NA - 1))
                                    first = False
                # z1 = tanh(0.666*pre + 0.666*bb1)
                def split_phase(dst, dstf, src_ps, scale):
                    # dst: [128, M, 2*BC] hi/lo bf16; dstf: [128, M*BC] f32
                    M = dst.shape[1]
                    dst3 = dst.rearrange("p m (a b) -> p m a b", a=NA)
                    nc.scalar.activation(dstf[:], src_ps[:], AF.Tanh, scale=scale)
                    dfv = dstf.rearrange("p (m b) -> p m b", m=M)
                    nc.scalar.activation(dst3[:, :, 0, :], dfv[:], AF.Copy)
                    nc.vector.tensor_sub(dst3[:, :, 1, :], dfv[:], dst3[:, :, 0, :])

                if chain2 and not cfg["ko_act"]:
                    z1fl = z1.rearrange("p m b -> p (m b)")
                    nc.scalar.activation(z1fl[:, 0:H1 * BC], z1psA[:],
                                         AF.Tanh, scale=0.666)
                    nc.scalar.activation(z1fl[:, H1 * BC:M1 * BC], z1psB[:],
                                         AF.Tanh, scale=0.666)
                elif chain and not cfg["ko_act"]:
                    nc.scalar.activation(z1.rearrange("p m b -> p (m b)"),
                                         z1ps[:], AF.Tanh, scale=0.666)
                elif cfg["preu"] and not cfg["ko_act"]:
                    zpre = ap_.tile([128, M1 * BC], f32, tag="zpre")
                    nc.vector.scalar_tensor_tensor(
                        zpre[:], z1ps[:], 0.666, u_sb[:, ds(t_idx, 1), :],
                        mybir.AluOpType.mult, mybir.AluOpType.add)
                    if cfg["asplit"]:
                        split_phase(z1, z1f, zpre, 1.0)
                    else:
                        nc.scalar.activation(z1.rearrange("p m b -> p (m b)"),
                                             zpre[:], AF.Tanh)
                elif cfg["wide"] and not cfg["ko_act"]:
                    if cfg["asplit"]:
                        split_phase(z1, z1f, z1ps, 0.666)
                    else:
                        nc.scalar.activation(z1.rearrange("p m b -> p (m b)"),
                                             z1ps[:], AF.Tanh, scale=0.666)
                else:
                    for m in range(M1 if not cfg["ko_act"] else 0):
                        nc.scalar.activation(z1[:, m, :], z1ps[:, ts(m, BC)],
                                             AF.Tanh, bias=bia[:, m:m + 1], scale=0.666)

                # ---- mm2: z2pre = z1 @ (1.7159*wb2).T
                if chain2 and not cfg["ko_mm"]:
                    for hf, zp in ((0, z2psA), (1, z2psB)):
                        for m in range(H2):
                            for k in range(K2):
                                nc.tensor.matmul(
                                    zp[:, ts(m, BC)],
                                    w2[:, 0, k, ts(hf * H2 + m, 128)],
                                    z1[:, k if not cfg["ko_act"] else k % KH,
                                       0:BC],
                                    start=(m == 0 and k == 0),
                                    stop=(m == H2 - 1 and k == K2 - 1))
                else:
                    for m in range(M2 if not cfg["ko_mm"] else 0):
                        for p in range(NW):
                            for k in range(K2):
                                for a in range(NA):
                                    nc.tensor.matmul(
                                        z2ps[:, ts(m, BC)],
                                        w2[:, p, k, ts(m, 128)],
                                        z1[:, k if not cfg["ko_act"] else k % KH,
                                           ts(a, BC)],
                                        start=(p == 0 and k == 0 and a == 0),
                                        stop=(p == NW - 1 and k == K2 - 1
                                              and a == NA - 1))
                if chain2 and not cfg["ko_act"]:
                    z2fl = z2.rearrange("p m b -> p (m b)")
                    nc.scalar.activation(z2fl[:, 0:H2 * BC], z2psA[:],
                                         AF.Tanh, scale=0.666)
                    nc.scalar.activation(z2fl[:, H2 * BC:M2 * BC], z2psB[:],
                                         AF.Tanh, scale=0.666)
                elif cfg["wide"] and not cfg["ko_act"]:
                    if cfg["asplit"]:
                        split_phase(z2, z2f, z2ps, 0.666)
                    else:
                        nc.scalar.activation(z2.rearrange("p m b -> p (m b)"),
                                             z2ps[:], AF.Tanh, scale=0.666)
                else:
                    for m in range(M2 if not cfg["ko_act"] else 0):
                        nc.scalar.activation(z2[:, m, :], z2ps[:, ts(m, BC)],
                                             AF.Tanh, bias=bia[:, 8 + m:9 + m], scale=0.666)

                # ---- heads: ff1, ff2, t (weights pre-scaled by 1.7159)
                # chain: emit tt first so its ACT overlaps the ff1/ff2 matmuls
                # and the DVE tail starts right after ff2's ACT.
                head_seq = ((2, wt), (0, wf1), (1, wf2)) if chain else \
                           ((0, wf1), (1, wf2), (2, wt))
                for hd, w_sb in (head_seq if not cfg["ko_mm"] else ()):
                    for m in range(MH):
                        for p in range(NW):
                            for k in range(K2):
                                for a in range(NA):
                                    dst = (tps[:, ts(m, BC)]
                                           if (chain and hd == 2) else
                                           hps[:, ts(hd * MH + m, BC)])
                                    nc.tensor.matmul(
                                        dst,
                                        w_sb[:, p, k, ts(m, 128)],
                                        z2[:, k if not cfg["ko_act"] else k % KH,
                                           ts(a, BC)],
                                        start=(p == 0 and k == 0 and a == 0),
                                        stop=(p == NW - 1 and k == K2 - 1
                                              and a == NA - 1))
                if cfg["wide"] and not cfg["ko_act"]:
                    if chain2:
                        nc.scalar.activation(tt[:], tps[:], AF.Tanh, scale=0.5)
                        nc.scalar.activation(ff12[:], hps[:], AF.Tanh)
                    elif chain:
                        nc.scalar.activation(tt[:], tps[:], AF.Tanh, scale=0.5)
                        nc.scalar.activation(ff1[:], hps[:, 0:MH * BC], AF.Tanh)
                        nc.scalar.activation(ff2[:], hps[:, MH * BC:2 * MH * BC],
                                             AF.Tanh)
                    else:
                        # ff1|ff2 in one tanh over hps cols [0, 2*MH*BC)
                        nc.scalar.activation(ff1[:], hps[:, 0:MH * BC], AF.Tanh)
                        nc.scalar.activation(ff2[:], hps[:, MH * BC:2 * MH * BC],
                                             AF.Tanh)
                        nc.scalar.activation(tt[:], hps[:, 2 * MH * BC:3 * MH * BC],
                                             AF.Tanh, scale=0.5)
                else:
                    for m in range(MH if not cfg["ko_act"] else 0):
                        nc.scalar.activation(ff1[:, ts(m, BC)], hps[:, ts(m, BC)],
                                             AF.Tanh, bias=bia[:, 16 + m:17 + m])
                    for m in range(MH if not cfg["ko_act"] else 0):
                        nc.scalar.activation(ff2[:, ts(m, BC)], hps[:, ts(MH + m, BC)],
                                             AF.Tanh, bias=bia[:, 20 + m:21 + m])
                    for m in range(MH if not cfg["ko_act"] else 0):
                        if cfg["onetable"]:
                            nc.scalar.activation(tt[:, ts(m, BC)], hps[:, ts(2 * MH + m, BC)],
                                                 AF.Tanh, bias=bia[:, 24 + m:25 + m],
                                                 scale=0.5)
                        else:
                            nc.scalar.activation(tt[:, ts(m, BC)], hps[:, ts(2 * MH + m, BC)],
                                                 AF.Sigmoid, bias=bia[:, 24 + m:25 + m])

                # ---- h' = ff1 + t*(ff2-ff1); onetable: t = 0.5*(1+tt_raw)
                if cfg["wide"] and not cfg["ko_act"]:
                    W = MH * BC
                    d = ap_.tile([128, W], f32, tag="d")
                    e = ap_.tile([128, W], f32, tag="e")
                    f1 = ff12[:, 0:W] if chain2 else ff1[:]
                    f2 = ff12[:, W:2 * W] if chain2 else ff2[:]
                    # e first: depends only on tt, overlaps the ff1/ff2 ACTs
                    nc.vector.tensor_scalar_add(e[:], tt[:], 1.0)
                    nc.vector.tensor_sub(d[:], f2, f1)
                    nc.vector.tensor_mul(d[:], d[:], e[:])
                    nc.vector.scalar_tensor_tensor(
                        h32[:], d[:], 0.5, f1,
                        mybir.AluOpType.mult, mybir.AluOpType.add)
                for c in range(0 if (cfg["wide"] or cfg["ko_act"]) else KH):
                    d = ap_.tile([128, BC], f32, tag="d")
                    e = ap_.tile([128, BC], f32, tag="e")
                    nc.vector.tensor_sub(d[:], ff2[:, ts(c, BC)], ff1[:, ts(c, BC)])
                    if cfg["onetable"]:
                        nc.vector.tensor_scalar_add(e[:], tt[:, ts(c, BC)], 1.0)
                        nc.vector.tensor_mul(d[:], d[:], e[:])
                        nc.vector.scalar_tensor_tensor(
                            h32[:, ts(c, BC)], d[:], 0.5, ff1[:, ts(c, BC)],
                            mybir.AluOpType.mult, mybir.AluOpType.add)
                    else:
                        nc.vector.tensor_mul(e[:], d[:], tt[:, ts(c, BC)])
                        nc.vector.tensor_add(h32[:, ts(c, BC)], e[:], ff1[:, ts(c, BC)])
                if not cfg["ko_act"] and (DT != f32 or cfg["asplit"]):
                    h3 = h.rearrange("p c (a b) -> p c a b", a=NA)
                    h32v = h32.rearrange("p (c b) -> p c b", c=KH)
                    if chain:
                        # DVE copy keeps the tail on one engine (ACT hop saved)
                        nc.vector.tensor_copy(h3[:, :, 0, :], h32v[:])
                    else:
                        nc.scalar.activation(h3[:, :, 0, :], h32v[:], AF.Copy)
                    if cfg["asplit"]:
                        nc.vector.tensor_sub(h3[:, :, 1, :], h32v[:], h3[:, :, 0, :])

                if not cfg["ko_dma"]:
                    if ys16:
                        nc.sync.dma_start(ys_d[ds(t_idx, 1), :, :],
                                          h.rearrange("p c b -> p (c b)")[:])
                    else:
                        nc.sync.dma_start(ys_d[ds(t_idx, 1), :, :], h32[:])

            U = cfg["unroll"]
            hint = ()
            if cfg["hints"]:
                hint = (mybir.EngineType.PE, mybir.EngineType.Activation,
                        mybir.EngineType.DVE)

            def t_loop():
                nc.vector.memset(h[:], 0.0)
                if DT != f32 or cfg["asplit"]:
                    nc.vector.memset(h32[:], 0.0)
                if cfg.get("static"):
                    for i in range(0, T_steps, U):
                        for u in range(U):
                            step(i + u)
                else:
                    with tc.For_i(0, T_steps, U, hint_engines=hint,
                                  staggered_reset=cfg["staggered"]) as i:
                        for u in range(U):
                            step(i + u if u else i)

            if cfg["outer"] == 1:
                t_loop()
            else:
                with tc.For_i(0, cfg["outer"], 1):
                    t_loop()

    nc.compile()
    return nc


# --- host side -------------------------------------------------------------

def _chunk(w2d):
    """[K, M] row-chunked to [128, K//128, M]."""
    K, M = w2d.shape
    return np.ascontiguousarray(
        w2d.reshape(K // 128, 128, M).transpose(1, 0, 2))


def _wprep(w2d, np_dt_w, wsplit):
    """[K, M] -> [128, NW, K//128, M] in np_dt_w (hi/lo split if wsplit)."""
    c = _chunk(w2d.astype(np.float32))
    hi = c.astype(np_dt_w)
    if not wsplit:
        return hi[:, None]
    lo = (c - hi.astype(np.float32)).astype(np_dt_w)
    return np.ascontiguousarray(np.stack([hi, lo], axis=1))


def _prep(np_dt_w, np_dt_a, x, wb1, bb1, wb2, bb2, wff1, bff1, wff2, bff2, wta, bta, wtb, btb,
          cfg=None):
    cfg = cfg or CFG
    f32 = np.float32
    ws = cfg["wsplit"]
    w1 = _wprep(wb1.T, np_dt_w, ws)                       # [128, NW, 6, 1024]
    w2 = _wprep((1.7159 * wb2).T, np_dt_w, ws)
    wf1 = _wprep((1.7159 * wff1).T, np_dt_w, ws)
    wf2 = _wprep((1.7159 * wff2).T, np_dt_w, ws)
    wt = _wprep((1.7159 * (wta + wtb)).T, np_dt_w, ws)
    bias = np.zeros((128, 28), f32)
    bias[:, 0:8] = (0.666 * bb1).reshape(8, 128).T
    bias[:, 8:16] = (0.666 * bb2).reshape(8, 128).T
    bias[:, 16:20] = bff1.reshape(4, 128).T
    bias[:, 20:24] = bff2.reshape(4, 128).T
    bias[:, 24:28] = (0.5 if cfg['onetable'] else 1.0) * (bta + btb).reshape(4, 128).T

    wag = cfg.get("wag", False)
    if wag:
        shards = []
        for c in range(NCORES):
            parts = []
            for arr, cols in ((w1, 6 * BU), (w2, K2 * BU), (wf1, K2 * H),
                              (wf2, K2 * H), (wt, K2 * H)):
                flat = arr.reshape(128, cols)
                sl = cols // NCORES
                parts.append(flat[:, c * sl:(c + 1) * sl])
            shards.append(np.ascontiguousarray(np.concatenate(parts, axis=1)))

    in_maps = []
    for c in range(NCORES):
        xc = x[c * BC:(c + 1) * BC].astype(f32)                     # [BC, T, I]
        xTf = np.ascontiguousarray(
            xc.reshape(BC, T, KX, 128).transpose(3, 1, 2, 0))       # [128,T,KX,BC]
        hi = xTf.astype(np_dt_a)
        if cfg["asplit"]:
            lo = (xTf - hi.astype(f32)).astype(np_dt_a)
            xT = np.ascontiguousarray(
                np.stack([hi, lo], axis=3)).reshape(128, T, KX, 2 * BC)
        else:
            xT = hi
        if wag:
            m = dict(xT=xT, wsh=shards[c], biases=bias)
        else:
            m = dict(xT=xT, w1=w1, w2=w2, wf1=wf1, wf2=wf2, wt=wt, biases=bias)
        if (cfg.get("chain", False) and cfg["preu"] and cfg["wide"]
                and not cfg["asplit"] and not cfg["wsplit"]):
            m["id128"] = np.eye(128, dtype=np_dt_w)
        in_maps.append(m)
    return in_maps


_CACHE = {}
_RUNNER_CACHE = {}
LAST_EXEC_NS = None


def make_runner(nc, n_cores=NCORES):
    """jit-compiled SPMD runner with on-device zero output buffers.

    Returns (fn, in_names, out_names): fn takes per-core input dicts
    (list of n_cores dicts of np/jax arrays) and returns a list of global
    jax output arrays (concat on axis 0 across cores).
    """
    import jax
    import jax.numpy as jnp
    from jax.experimental.shard_map import shard_map
    from jax.sharding import Mesh, PartitionSpec
    from concourse.bass2jax import (_bass_exec_p, install_neuronx_cc_hook,
                                    partition_id_tensor)

    install_neuronx_cc_hook()
    partition_name = nc.partition_id_tensor.name if nc.partition_id_tensor else None
    in_names, out_names, out_avals = [], [], []
    for alloc in nc.m.functions[0].allocations:
        if not isinstance(alloc, mybir.MemoryLocationSet):
            continue
        name = alloc.memorylocations[0].name
        if alloc.kind == "ExternalInput":
            if name != partition_name:
                in_names.append(name)
        elif alloc.kind == "ExternalOutput":
            out_names.append(name)
            out_avals.append(
                (tuple(alloc.tensor_shape), mybir.dt.np(alloc.dtype)))
    all_in = list(in_names) + list(out_names)
    if partition_name is not None:
        all_in.append(partition_name)
    import jax.core as jcore
    avals = tuple(jcore.ShapedArray(s, d) for s, d in out_avals)

    def _body(*args):
        operands = list(args)
        if partition_name is not None:
            operands.append(partition_id_tensor())
        outs = _bass_exec_p.bind(
            *operands,
            out_avals=avals,
            in_names=tuple(all_in),
            out_names=tuple(out_names),
            lowering_input_output_aliases=(),
            sim_require_finite=True,
            sim_require_nnan=True,
            nc=nc,
        )
        return tuple(outs)

    devices = jax.devices()[:n_cores]
    mesh = Mesh(np.asarray(devices), ("core",))
    from jax.sharding import NamedSharding
    sharding = NamedSharding(mesh, PartitionSpec("core"))
    n_params, n_outs = len(in_names), len(out_names)
    jf = jax.jit(
        shard_map(
            _body, mesh=mesh,
            in_specs=(PartitionSpec("core"),) * (n_params + n_outs),
            out_specs=(PartitionSpec("core"),) * n_outs,
            check_rep=False),
        donate_argnums=tuple(range(n_params, n_params + n_outs)))
    # on-device zero output buffers (no host->device transfer)
    zmk = jax.jit(
        lambda: tuple(jnp.zeros((n_cores * s[0], *s[1:]), d) for s, d in out_avals),
        out_shardings=(sharding,) * n_outs)

    def fn(in_maps):
        concat_in = [
            np.concatenate([np.asarray(in_maps[c][n]) for c in range(n_cores)],
                           axis=0)
            for n in in_names
        ]
        return jf(*concat_in, *zmk())

    fn.jit = jf
    fn.make_zeros = zmk
    fn.sharding = sharding
    return fn, in_names, out_names


def kernel(**inputs):
    import ml_dtypes
    inputs = {k: np.asarray(v) for k, v in inputs.items()}
    # wide mode folds biases away; only valid when all biases are zero
    if (CFG["wide"] or CFG["preu"]) and any(
            np.any(np.asarray(inputs[k]) != 0)
            for k in ("bb1", "bb2", "bff1", "bff2", "bta", "btb")):
        CFG["wide"] = False
        CFG["preu"] = False

    def npdt(s):
        return {"float32": np.float32, "bfloat16": ml_dtypes.bfloat16}[s]

    key = tuple(sorted(CFG.items()))
    if key not in _CACHE:
        _CACHE[key] = build(T, CFG)
        _RUNNER_CACHE[key] = make_runner(_CACHE[key])
    fn, in_names, out_names = _RUNNER_CACHE[key]
    in_maps = _prep(npdt(CFG["dtype_w"]), npdt(CFG["dtype_a"]), cfg=CFG, **inputs)
    outs = fn(in_maps)
    ys_g = np.asarray(outs[out_names.index("ys")])   # [NC*T, 128, KH*BC]
    ys_g = ys_g.reshape(NCORES, T, 128, KH, BC).astype(np.float32)
    out = np.ascontiguousarray(
        ys_g.transpose(0, 4, 1, 3, 2).reshape(B, T, H))
    return out

